# revision 14
# baseline (speedup 1.0000x reference)
"""HardClusterAssigner Trainium2 kernel.

Reference computation:
    x_emb = mean_b(einsum('bsv,hs->bvh', x, W) + b)   # [V, H]
    assignments = one_hot(argmin(-l2norm(x_emb) @ l2norm(centroids).T))

Key transformations:
  1. mean over B commutes with the linear contraction over S, the l2norm of
     the embedding is a positive per-row scale (argmin-invariant), and the
     1/B + bias fold in exactly:
         sim[v,c] = (sum_b x)[s,v] @ M[s,c] + bn[c],
         M = W.T @ cn.T,  bn = B * (b @ cn.T),  cn = l2norm(centroids)
     M/bn are x-independent and folded on the host (fp64), shipped as exact
     fp16 hi+lo pairs (pair error ~1e-7 relative).
  2. x streams as fp16 (halves the dominant HBM traffic) in [s, b, v]
     layout as 16 half-chunk tiles, alternating between both HWDGE rings
     (SP + ACT). Each tile's b-reduction runs as a 2-level halving add tree
     on the DVE over fully contiguous 2D slices (fp16 2x packed mode),
     leaving 8 slabs per tile; the PE contracts the remaining (s, slab)
     axes with fp16xfp16 products accumulated exactly in fp32 PSUM (M
     hi/lo stationary). Verified argmax-exact on the reference inputs with
     a 2.2e-3 worst-row margin (~100x the device-vs-host numeric noise).
  3. sim lands PSUM-transposed as two [c, (slab v)] banks; slab-reduces +
     identity-matmul transposes (exact: multiplies by 1.0/0.0) accumulate
     [v, c] into one PSUM tile for the row-max + is_equal one-hot.

Sharding: V is split across the 8 cores; every stage after the split is
core-local (no collectives). Per-core time is DMA-bound: ~8.9 MB/core
(x 8.4 MB fp16 + M 0.26 MB) at the ~358 GB/s HBM roofline, with the DVE
trees (~17us) and all PE work hidden under the stream.
"""

import sys

for _p in ("/opt/trn_rl_repo",):
    if _p not in sys.path:
        sys.path.append(_p)

from contextlib import ExitStack

import numpy as np

import concourse.bacc as bacc
import concourse.bass as bass
import concourse.mybir as mybir
from concourse import tile
from concourse.bass_utils import run_bass_kernel_spmd

B, S, V, H, C = 64, 1024, 512, 512, 64
NCORES = 8
VL = V // NCORES  # 64 V-columns per core
P = 128
ST = S // P  # 8 s-chunks
FH = (B // 2) * VL  # 2048 free elems per half-chunk tile
JH = 8  # slabs per half-tile left for the PE after the 2-level DVE tree
F16 = mybir.dt.float16
F32 = mybir.dt.float32

_NC_CACHE = None


def build_bass() -> bass.Bass:
    nc = bacc.Bacc("TRN2", target_bir_lowering=False)

    xs = nc.declare_dram_parameter("xs", [S, 2 * FH], F16, isOutput=False)
    mm = nc.declare_dram_parameter("mm", [P, 2 * ST * C], F16, isOutput=False)
    c16 = nc.declare_dram_parameter("c16", [1, VL + 2 * C], F16, isOutput=False)
    c32 = nc.declare_dram_parameter("c32", [C, C], F32, isOutput=False)
    out = nc.declare_dram_parameter("out", [VL, C], F32, isOutput=True)

    with tile.TileContext(nc) as tc, ExitStack() as ctx:
        consts = ctx.enter_context(tc.tile_pool(name="consts", bufs=1))
        xpool = ctx.enter_context(tc.tile_pool(name="x", bufs=16))
        spool = ctx.enter_context(tc.tile_pool(name="small", bufs=1))
        pst = ctx.enter_context(tc.tile_pool(name="pst", bufs=1, space="PSUM"))
        psca = ctx.enter_context(tc.tile_pool(name="psca", bufs=1, space="PSUM"))
        pscb = ctx.enter_context(tc.tile_pool(name="pscb", bufs=1, space="PSUM"))

        # consts ride the ACT ring ahead of its share of the x stream;
        # flat 2D transfers only
        msb = consts.tile([P, 2 * ST * C], F16)
        nc.scalar.dma_start(out=msb[:], in_=mm[:])
        c16t = consts.tile([1, VL + 2 * C], F16)
        nc.scalar.dma_start(out=c16t[:], in_=c16[:])
        idt = consts.tile([C, C], F32)
        nc.scalar.dma_start(out=idt[:], in_=c32[:])

        # final sim [v, c]; two PE-side accumulators [c, (slab v)]
        psT = pst.tile([VL, C], F32, tag="psT")
        psCa = psca.tile([C, JH * VL], F32, tag="psCa")
        psCb = pscb.tile([C, JH * VL], F32, tag="psCb")
        psC = [psCa, psCb]

        xs_r = xs.rearrange("(t p) f -> t p f", p=P)
        engs = [nc.sync, nc.scalar]
        for t in range(ST):
            for h in range(2):
                xh = xpool.tile([P, FH], F16, tag="xh")
                engs[(2 * t + h) % 2].dma_start(
                    out=xh[:], in_=xs_r[t][:, h * FH : (h + 1) * FH]
                )
                # 2-level halving tree over contiguous column blocks
                # (fp16 2x mode); cols = b_local*VL + v
                nb = FH
                while nb > JH * VL:
                    hb = nb // 2
                    nc.vector.tensor_tensor(
                        xh[:, 0:hb], xh[:, 0:hb], xh[:, hb:nb],
                        op=mybir.AluOpType.add,
                    )
                    nb = hb
                # slab contraction, M_t hi/lo stationary (flat col slices)
                for li in range(2):
                    nc.tensor.matmul(
                        psC[h][:],
                        msb[:, (li * ST + t) * C : (li * ST + t + 1) * C],
                        xh[:, 0 : JH * VL],
                        start=(t == 0 and li == 0),
                        stop=(t == ST - 1 and li == 1),
                    )
            if t == 0:
                # bias early, off the critical path (psT group start)
                ones16 = c16t[0:1, 0:VL]
                nc.tensor.matmul(
                    psT[:], ones16, c16t[0:1, VL : VL + C],
                    start=True, stop=False,
                )
                nc.tensor.matmul(
                    psT[:], ones16, c16t[0:1, VL + C : VL + 2 * C],
                    start=False, stop=False,
                )

        # --- tail: slab-reduces, transposes into [v, c], argmax ------------
        sCa = spool.tile([C, VL], F32, tag="sCa")
        sCb = spool.tile([C, VL], F32, tag="sCb")
        sC = [sCa, sCb]
        for h in range(2):
            nc.vector.tensor_reduce(
                sC[h][:],
                psC[h][:].rearrange("c (s v) -> c v s", s=JH),
                axis=mybir.AxisListType.X,
                op=mybir.AluOpType.add,
            )
        nc.tensor.matmul(psT[:], sC[0][:], idt[:], start=False, stop=False)
        nc.tensor.matmul(psT[:], sC[1][:], idt[:], start=False, stop=True)

        mx = spool.tile([VL, 1], F32)
        nc.vector.tensor_reduce(
            mx[:], psT[:], axis=mybir.AxisListType.X, op=mybir.AluOpType.max
        )
        oh = spool.tile([VL, C], F32)
        nc.vector.tensor_scalar(
            oh[:], psT[:], mx[:], None, op0=mybir.AluOpType.is_equal
        )
        nc.sync.dma_start(out=out[:], in_=oh[:])

    nc.compile()
    return nc


def _get_nc() -> bass.Bass:
    global _NC_CACHE
    if _NC_CACHE is None:
        _NC_CACHE = build_bass()
    return _NC_CACHE


def make_in_maps(x, W, b, centroids):
    x = np.asarray(x, dtype=np.float32)
    W = np.asarray(W, dtype=np.float32)
    b = np.asarray(b, dtype=np.float32)
    centroids = np.asarray(centroids, dtype=np.float32)

    # x-independent folds, in float64, shipped as exact fp16 hi+lo pairs
    cn = centroids.astype(np.float64)
    cn /= np.linalg.norm(cn, axis=1, keepdims=True)
    M = W.astype(np.float64).T @ cn.T  # [S, C]
    bn = np.float64(B) * (b.astype(np.float64) @ cn.T)  # [C]

    Mhi = M.astype(np.float16)
    Mlo = (M - Mhi.astype(np.float64)).astype(np.float16)
    mhost = np.empty((P, 2, ST, C), np.float16)
    mhost[:, 0] = Mhi.reshape(ST, P, C).transpose(1, 0, 2)
    mhost[:, 1] = Mlo.reshape(ST, P, C).transpose(1, 0, 2)
    mhost = np.ascontiguousarray(mhost).reshape(P, 2 * ST * C)

    bnhi = bn.astype(np.float16)
    bnlo = (bn - bnhi.astype(np.float64)).astype(np.float16)
    c16host = np.concatenate(
        [np.ones(VL, np.float16), bnhi, bnlo]
    ).reshape(1, VL + 2 * C)
    c32host = np.eye(C, dtype=np.float32)

    # Host layout [B,S,V] -> [S, B, VL] per core, in fp16 (cast first so the
    # transpose moves half the bytes). One pass to [S, B, V] (contiguous 1KB
    # runs), then a contiguous per-core V-slice.
    x16 = x.astype(np.float16)
    xsb = np.ascontiguousarray(x16.transpose(1, 0, 2))  # [S, B, V]
    in_maps = []
    for i in range(NCORES):
        xs_i = np.ascontiguousarray(
            xsb[:, :, i * VL : (i + 1) * VL]
        ).reshape(S, 2 * FH)
        in_maps.append(
            {"xs": xs_i, "mm": mhost, "c16": c16host, "c32": c32host}
        )
    return in_maps


def run(inputs: dict, trace: bool = False):
    """Run on the 8 NeuronCores; returns (full_output, BassKernelResults)."""
    nc = _get_nc()
    in_maps = make_in_maps(**inputs)
    res = run_bass_kernel_spmd(nc, in_maps, list(range(NCORES)), trace=trace)
    full = np.concatenate([r["out"] for r in res.results], axis=0)
    return full, res


def kernel(x, W, b, centroids) -> np.ndarray:
    full, _ = run({"x": x, "W": W, "b": b, "centroids": centroids})
    return full


# revision 16
# speedup vs baseline: 1.0556x; 1.0556x over previous
"""HardClusterAssigner Trainium2 kernel.

Reference computation:
    x_emb = mean_b(einsum('bsv,hs->bvh', x, W) + b)   # [V, H]
    assignments = one_hot(argmin(-l2norm(x_emb) @ l2norm(centroids).T))

Key transformations:
  1. mean over B commutes with the linear contraction over S, the l2norm of
     the embedding is a positive per-row scale (argmin-invariant), and the
     1/B + bias fold in exactly:
         sim[v,c] = (sum_b x)[s,v] @ M[s,c] + bn[c],
         M = W.T @ cn.T,  bn = B * (b @ cn.T),  cn = l2norm(centroids)
     M/bn are x-independent and folded on the host (fp64), shipped as exact
     fp16 hi+lo pairs (pair error ~1e-7 relative).
  2. x streams as fp16 (halves the dominant HBM traffic) in [s, b, v]
     layout as 16 half-chunk tiles on the SP HWDGE ring (consts ride the
     ACT ring in parallel). Each tile's b-reduction runs as a 2-level add tree
     on the DVE over fully contiguous 2D slices (fp16 2x packed mode),
     leaving 8 slabs per tile; the PE contracts the remaining (s, slab)
     axes with fp16xfp16 products accumulated exactly in fp32 PSUM (M
     hi/lo stationary). Verified argmax-exact on the reference inputs with
     a 2.2e-3 worst-row margin (~100x the device-vs-host numeric noise).
  3. sim lands PSUM-transposed as two [c, (slab v)] banks; slab-reduces +
     identity-matmul transposes (exact: multiplies by 1.0/0.0) accumulate
     [v, c] into one PSUM tile for the row-max + is_equal one-hot.

Sharding: V is split across the 8 cores; every stage after the split is
core-local (no collectives). Per-core time is DMA-bound: ~8.9 MB/core
(x 8.4 MB fp16 + M 0.26 MB) at the ~358 GB/s HBM roofline, with the DVE
trees (~17us) and all PE work hidden under the stream.
"""

import sys

for _p in ("/opt/trn_rl_repo",):
    if _p not in sys.path:
        sys.path.append(_p)

from contextlib import ExitStack

import numpy as np

import concourse.bacc as bacc
import concourse.bass as bass
import concourse.mybir as mybir
from concourse import tile
from concourse.bass_utils import run_bass_kernel_spmd

B, S, V, H, C = 64, 1024, 512, 512, 64
NCORES = 8
VL = V // NCORES  # 64 V-columns per core
P = 128
ST = S // P  # 8 s-chunks
FH = (B // 2) * VL  # 2048 free elems per half-chunk tile
JH = 8  # slabs per half-tile left for the PE after the 2-level DVE tree
F16 = mybir.dt.float16
F32 = mybir.dt.float32

_NC_CACHE = None


def build_bass() -> bass.Bass:
    nc = bacc.Bacc("TRN2", target_bir_lowering=False)

    xs = nc.declare_dram_parameter("xs", [S, 2 * FH], F16, isOutput=False)
    mm = nc.declare_dram_parameter("mm", [P, 2 * ST * C], F16, isOutput=False)
    c16 = nc.declare_dram_parameter("c16", [1, VL + 2 * C], F16, isOutput=False)
    c32 = nc.declare_dram_parameter("c32", [C, C], F32, isOutput=False)
    out = nc.declare_dram_parameter("out", [VL, C], F32, isOutput=True)

    with tile.TileContext(nc) as tc, ExitStack() as ctx:
        consts = ctx.enter_context(tc.tile_pool(name="consts", bufs=1))
        xpool = ctx.enter_context(tc.tile_pool(name="x", bufs=16))
        spool = ctx.enter_context(tc.tile_pool(name="small", bufs=1))
        pst = ctx.enter_context(tc.tile_pool(name="pst", bufs=1, space="PSUM"))
        psca = ctx.enter_context(tc.tile_pool(name="psca", bufs=1, space="PSUM"))
        pscb = ctx.enter_context(tc.tile_pool(name="pscb", bufs=1, space="PSUM"))

        # consts ride the ACT ring ahead of its share of the x stream;
        # flat 2D transfers only
        msb = consts.tile([P, 2 * ST * C], F16)
        nc.scalar.dma_start(out=msb[:], in_=mm[:])
        c16t = consts.tile([1, VL + 2 * C], F16)
        nc.scalar.dma_start(out=c16t[:], in_=c16[:])
        idt = consts.tile([C, C], F32)
        nc.scalar.dma_start(out=idt[:], in_=c32[:])

        # final sim [v, c]; two PE-side accumulators [c, (slab v)]
        psT = pst.tile([VL, C], F32, tag="psT")
        psCa = psca.tile([C, JH * VL], F32, tag="psCa")
        psCb = pscb.tile([C, JH * VL], F32, tag="psCb")
        psC = [psCa, psCb]

        xs_r = xs.rearrange("(t p) f -> t p f", p=P)
        for t in range(ST):
            for h in range(2):
                xh = xpool.tile([P, FH], F16, tag="xh")
                nc.sync.dma_start(
                    out=xh[:], in_=xs_r[t][:, h * FH : (h + 1) * FH]
                )
                # 2-level halving tree over contiguous column blocks
                # (fp16 2x mode); cols = b_local*VL + v
                nb = FH
                while nb > JH * VL:
                    hb = nb // 2
                    nc.vector.tensor_tensor(
                        xh[:, 0:hb], xh[:, 0:hb], xh[:, hb:nb],
                        op=mybir.AluOpType.add,
                    )
                    nb = hb
                # slab contraction, M_t hi/lo stationary (flat col slices)
                for li in range(2):
                    nc.tensor.matmul(
                        psC[h][:],
                        msb[:, (li * ST + t) * C : (li * ST + t + 1) * C],
                        xh[:, 0 : JH * VL],
                        start=(t == 0 and li == 0),
                        stop=(t == ST - 1 and li == 1),
                    )
            if t == 0:
                # bias early, off the critical path (psT group start)
                ones16 = c16t[0:1, 0:VL]
                nc.tensor.matmul(
                    psT[:], ones16, c16t[0:1, VL : VL + C],
                    start=True, stop=False,
                )
                nc.tensor.matmul(
                    psT[:], ones16, c16t[0:1, VL + C : VL + 2 * C],
                    start=False, stop=False,
                )

        # --- tail: slab-reduces, transposes into [v, c], argmax ------------
        sCa = spool.tile([C, VL], F32, tag="sCa")
        sCb = spool.tile([C, VL], F32, tag="sCb")
        sC = [sCa, sCb]
        for h in range(2):
            nc.vector.tensor_reduce(
                sC[h][:],
                psC[h][:].rearrange("c (s v) -> c v s", s=JH),
                axis=mybir.AxisListType.X,
                op=mybir.AluOpType.add,
            )
        nc.tensor.matmul(psT[:], sC[0][:], idt[:], start=False, stop=False)
        nc.tensor.matmul(psT[:], sC[1][:], idt[:], start=False, stop=True)

        mx = spool.tile([VL, 1], F32)
        nc.vector.tensor_reduce(
            mx[:], psT[:], axis=mybir.AxisListType.X, op=mybir.AluOpType.max
        )
        oh = spool.tile([VL, C], F32)
        nc.vector.tensor_scalar(
            oh[:], psT[:], mx[:], None, op0=mybir.AluOpType.is_equal
        )
        nc.sync.dma_start(out=out[:], in_=oh[:])

    nc.compile()
    return nc


def _get_nc() -> bass.Bass:
    global _NC_CACHE
    if _NC_CACHE is None:
        _NC_CACHE = build_bass()
    return _NC_CACHE


def make_in_maps(x, W, b, centroids):
    x = np.asarray(x, dtype=np.float32)
    W = np.asarray(W, dtype=np.float32)
    b = np.asarray(b, dtype=np.float32)
    centroids = np.asarray(centroids, dtype=np.float32)

    # x-independent folds, in float64, shipped as exact fp16 hi+lo pairs
    cn = centroids.astype(np.float64)
    cn /= np.linalg.norm(cn, axis=1, keepdims=True)
    M = W.astype(np.float64).T @ cn.T  # [S, C]
    bn = np.float64(B) * (b.astype(np.float64) @ cn.T)  # [C]

    Mhi = M.astype(np.float16)
    Mlo = (M - Mhi.astype(np.float64)).astype(np.float16)
    mhost = np.empty((P, 2, ST, C), np.float16)
    mhost[:, 0] = Mhi.reshape(ST, P, C).transpose(1, 0, 2)
    mhost[:, 1] = Mlo.reshape(ST, P, C).transpose(1, 0, 2)
    mhost = np.ascontiguousarray(mhost).reshape(P, 2 * ST * C)

    bnhi = bn.astype(np.float16)
    bnlo = (bn - bnhi.astype(np.float64)).astype(np.float16)
    c16host = np.concatenate(
        [np.ones(VL, np.float16), bnhi, bnlo]
    ).reshape(1, VL + 2 * C)
    c32host = np.eye(C, dtype=np.float32)

    # Host layout [B,S,V] -> [S, B, VL] per core, in fp16 (cast first so the
    # transpose moves half the bytes). One pass to [S, B, V] (contiguous 1KB
    # runs), then a contiguous per-core V-slice.
    x16 = x.astype(np.float16)
    xsb = np.ascontiguousarray(x16.transpose(1, 0, 2))  # [S, B, V]
    in_maps = []
    for i in range(NCORES):
        xs_i = np.ascontiguousarray(
            xsb[:, :, i * VL : (i + 1) * VL]
        ).reshape(S, 2 * FH)
        in_maps.append(
            {"xs": xs_i, "mm": mhost, "c16": c16host, "c32": c32host}
        )
    return in_maps


def run(inputs: dict, trace: bool = False):
    """Run on the 8 NeuronCores; returns (full_output, BassKernelResults)."""
    nc = _get_nc()
    in_maps = make_in_maps(**inputs)
    res = run_bass_kernel_spmd(nc, in_maps, list(range(NCORES)), trace=trace)
    full = np.concatenate([r["out"] for r in res.results], axis=0)
    return full, res


def kernel(x, W, b, centroids) -> np.ndarray:
    full, _ = run({"x": x, "W": W, "b": b, "centroids": centroids})
    return full


# revision 19
# speedup vs baseline: 1.1420x; 1.0818x over previous
"""HardClusterAssigner Trainium2 kernel.

Reference computation:
    x_emb = mean_b(einsum('bsv,hs->bvh', x, W) + b)   # [V, H]
    assignments = one_hot(argmin(-l2norm(x_emb) @ l2norm(centroids).T))

Key transformations:
  1. mean over B commutes with the linear contraction over S, the l2norm of
     the embedding is a positive per-row scale (argmin-invariant), and the
     1/B + bias fold in exactly:
         sim[v,c] = (sum_b x)[s,v] @ M[s,c] + bn[c],
         M = W.T @ cn.T,  bn = B * (b @ cn.T),  cn = l2norm(centroids)
     M/bn are x-independent and folded on the host (fp64), shipped as exact
     fp16 hi+lo pairs (pair error ~1e-7 relative).
  2. x streams as fp16 (halves the dominant HBM traffic) in [s, b, v]
     layout as 16 half-chunk tiles on the SP HWDGE ring (consts ride the
     ACT ring in parallel). Each tile's b-reduction runs as a 2-level add tree
     on the DVE over fully contiguous 2D slices (fp16 2x packed mode),
     leaving 8 slabs per tile; the PE contracts the remaining (s, slab)
     axes with fp16xfp16 products accumulated exactly in fp32 PSUM (M
     hi/lo stationary). Verified argmax-exact on the reference inputs with
     a 2.2e-3 worst-row margin (~100x the device-vs-host numeric noise).
  3. sim lands PSUM-transposed as two [c, (slab v)] banks; slab-reduces +
     identity-matmul transposes (exact: multiplies by 1.0/0.0) accumulate
     [v, c] into one PSUM tile for the row-max + is_equal one-hot.

Sharding: V is split across the 8 cores; every stage after the split is
core-local (no collectives). Per-core time is DMA-bound: ~8.9 MB/core
(x 8.4 MB fp16 + M 0.26 MB) at the ~358 GB/s HBM roofline, with the DVE
trees (~17us) and all PE work hidden under the stream.
"""

import sys

for _p in ("/opt/trn_rl_repo",):
    if _p not in sys.path:
        sys.path.append(_p)

from contextlib import ExitStack

import numpy as np

import concourse.bacc as bacc
import concourse.bass as bass
import concourse.mybir as mybir
from concourse import tile
from concourse.bass_utils import run_bass_kernel_spmd

B, S, V, H, C = 64, 1024, 512, 512, 64
NCORES = 8
VL = V // NCORES  # 64 V-columns per core
P = 128
ST = S // P  # 8 s-chunks
FH = (B // 2) * VL  # 2048 free elems per half-chunk tile
JH = 8  # slabs per half-tile left for the PE after the 2-level DVE tree
F16 = mybir.dt.float16
F32 = mybir.dt.float32

_NC_CACHE = None


def build_bass() -> bass.Bass:
    nc = bacc.Bacc("TRN2", target_bir_lowering=False)

    xs = nc.declare_dram_parameter("xs", [S, 2 * FH], F16, isOutput=False)
    mm = nc.declare_dram_parameter("mm", [P, 2 * ST * C], F16, isOutput=False)
    c16 = nc.declare_dram_parameter("c16", [1, VL + 2 * C], F16, isOutput=False)
    c32 = nc.declare_dram_parameter("c32", [C, C], F32, isOutput=False)
    out = nc.declare_dram_parameter("out", [VL, C], F32, isOutput=True)

    with tile.TileContext(nc) as tc, ExitStack() as ctx:
        consts = ctx.enter_context(tc.tile_pool(name="consts", bufs=1))
        xpool = ctx.enter_context(tc.tile_pool(name="x", bufs=16))
        spool = ctx.enter_context(tc.tile_pool(name="small", bufs=1))
        pst = ctx.enter_context(tc.tile_pool(name="pst", bufs=1, space="PSUM"))
        psca = ctx.enter_context(tc.tile_pool(name="psca", bufs=1, space="PSUM"))
        pscb = ctx.enter_context(tc.tile_pool(name="pscb", bufs=1, space="PSUM"))

        # const tiles; their DMAs ride the SP ring just behind the first
        # chunk (a second ring steals SDMA packet slots and slows the
        # stream). Flat 2D transfers only.
        msb = consts.tile([P, 2 * ST * C], F16)
        c16t = consts.tile([1, VL + 2 * C], F16)
        idt = consts.tile([C, C], F32)

        # final sim [v, c]; two PE-side accumulators [c, (slab v)]
        psT = pst.tile([VL, C], F32, tag="psT")
        psCa = psca.tile([C, JH * VL], F32, tag="psCa")
        psCb = pscb.tile([C, JH * VL], F32, tag="psCb")
        psC = [psCa, psCb]

        xs_r = xs.rearrange("(t p) f -> t p f", p=P)
        for t in range(ST):
            xhs = []
            for h in range(2):
                xh = xpool.tile([P, FH], F16, tag="xh")
                nc.sync.dma_start(
                    out=xh[:], in_=xs_r[t][:, h * FH : (h + 1) * FH]
                )
                # 2-level halving tree over contiguous column blocks
                # (fp16 2x mode); cols = b_local*VL + v
                nb = FH
                while nb > JH * VL:
                    hb = nb // 2
                    nc.vector.tensor_tensor(
                        xh[:, 0:hb], xh[:, 0:hb], xh[:, hb:nb],
                        op=mybir.AluOpType.add,
                    )
                    nb = hb
                xhs.append(xh)
            if t == 0:
                # consts land behind chunk 0 on the SP ring (a second ring
                # steals SDMA packet slots); the PE has slack to wait
                nc.sync.dma_start(out=msb[:], in_=mm[:])
                nc.sync.dma_start(out=c16t[:], in_=c16[:])
                nc.sync.dma_start(out=idt[:], in_=c32[:])
            # slab contraction, M_t hi/lo stationary (flat col slices)
            for h in range(2):
                for li in range(2):
                    nc.tensor.matmul(
                        psC[h][:],
                        msb[:, (li * ST + t) * C : (li * ST + t + 1) * C],
                        xhs[h][:, 0 : JH * VL],
                        start=(t == 0 and li == 0),
                        stop=(t == ST - 1 and li == 1),
                    )
            if t == 0:
                # bias early, off the critical path (psT group start)
                ones16 = c16t[0:1, 0:VL]
                nc.tensor.matmul(
                    psT[:], ones16, c16t[0:1, VL : VL + C],
                    start=True, stop=False,
                )
                nc.tensor.matmul(
                    psT[:], ones16, c16t[0:1, VL + C : VL + 2 * C],
                    start=False, stop=False,
                )

        # --- tail: slab-reduces, transposes into [v, c], argmax ------------
        sCa = spool.tile([C, VL], F32, tag="sCa")
        sCb = spool.tile([C, VL], F32, tag="sCb")
        sC = [sCa, sCb]
        for h in range(2):
            nc.vector.tensor_reduce(
                sC[h][:],
                psC[h][:].rearrange("c (s v) -> c v s", s=JH),
                axis=mybir.AxisListType.X,
                op=mybir.AluOpType.add,
            )
        nc.tensor.matmul(psT[:], sC[0][:], idt[:], start=False, stop=False)
        nc.tensor.matmul(psT[:], sC[1][:], idt[:], start=False, stop=True)

        mx = spool.tile([VL, 1], F32)
        nc.vector.tensor_reduce(
            mx[:], psT[:], axis=mybir.AxisListType.X, op=mybir.AluOpType.max
        )
        oh = spool.tile([VL, C], F32)
        nc.vector.tensor_scalar(
            oh[:], psT[:], mx[:], None, op0=mybir.AluOpType.is_equal
        )
        nc.sync.dma_start(out=out[:], in_=oh[:])

    nc.compile()
    return nc


def _get_nc() -> bass.Bass:
    global _NC_CACHE
    if _NC_CACHE is None:
        _NC_CACHE = build_bass()
    return _NC_CACHE


def make_in_maps(x, W, b, centroids):
    x = np.asarray(x, dtype=np.float32)
    W = np.asarray(W, dtype=np.float32)
    b = np.asarray(b, dtype=np.float32)
    centroids = np.asarray(centroids, dtype=np.float32)

    # x-independent folds, in float64, shipped as exact fp16 hi+lo pairs
    cn = centroids.astype(np.float64)
    cn /= np.linalg.norm(cn, axis=1, keepdims=True)
    M = W.astype(np.float64).T @ cn.T  # [S, C]
    bn = np.float64(B) * (b.astype(np.float64) @ cn.T)  # [C]

    Mhi = M.astype(np.float16)
    Mlo = (M - Mhi.astype(np.float64)).astype(np.float16)
    mhost = np.empty((P, 2, ST, C), np.float16)
    mhost[:, 0] = Mhi.reshape(ST, P, C).transpose(1, 0, 2)
    mhost[:, 1] = Mlo.reshape(ST, P, C).transpose(1, 0, 2)
    mhost = np.ascontiguousarray(mhost).reshape(P, 2 * ST * C)

    bnhi = bn.astype(np.float16)
    bnlo = (bn - bnhi.astype(np.float64)).astype(np.float16)
    c16host = np.concatenate(
        [np.ones(VL, np.float16), bnhi, bnlo]
    ).reshape(1, VL + 2 * C)
    c32host = np.eye(C, dtype=np.float32)

    # Host layout [B,S,V] -> [S, B, VL] per core, in fp16 (cast first so the
    # transpose moves half the bytes). One pass to [S, B, V] (contiguous 1KB
    # runs), then a contiguous per-core V-slice.
    x16 = x.astype(np.float16)
    xsb = np.ascontiguousarray(x16.transpose(1, 0, 2))  # [S, B, V]
    in_maps = []
    for i in range(NCORES):
        xs_i = np.ascontiguousarray(
            xsb[:, :, i * VL : (i + 1) * VL]
        ).reshape(S, 2 * FH)
        in_maps.append(
            {"xs": xs_i, "mm": mhost, "c16": c16host, "c32": c32host}
        )
    return in_maps


def run(inputs: dict, trace: bool = False):
    """Run on the 8 NeuronCores; returns (full_output, BassKernelResults)."""
    nc = _get_nc()
    in_maps = make_in_maps(**inputs)
    res = run_bass_kernel_spmd(nc, in_maps, list(range(NCORES)), trace=trace)
    full = np.concatenate([r["out"] for r in res.results], axis=0)
    return full, res


def kernel(x, W, b, centroids) -> np.ndarray:
    full, _ = run({"x": x, "W": W, "b": b, "centroids": centroids})
    return full


# revision 23
# speedup vs baseline: 1.1568x; 1.0130x over previous
"""HardClusterAssigner Trainium2 kernel.

Reference computation:
    x_emb = mean_b(einsum('bsv,hs->bvh', x, W) + b)   # [V, H]
    assignments = one_hot(argmin(-l2norm(x_emb) @ l2norm(centroids).T))

Key transformations:
  1. mean over B commutes with the linear contraction over S, the l2norm of
     the embedding is a positive per-row scale (argmin-invariant), and the
     1/B + bias fold in exactly:
         sim[v,c] = (sum_b x)[s,v] @ M[s,c] + bn[c],
         M = W.T @ cn.T,  bn = B * (b @ cn.T),  cn = l2norm(centroids)
     M/bn are x-independent and folded on the host (fp64), shipped as exact
     fp16 hi+lo pairs (pair error ~1e-7 relative).
  2. x streams as fp16 (halves the dominant HBM traffic) in [s, b, v]
     layout as 16 half-chunk tiles on the SP HWDGE ring (consts ride the
     ACT ring in parallel). Each tile's b-reduction runs as a 2-level add tree
     on the DVE over fully contiguous 2D slices (fp16 2x packed mode),
     leaving 8 slabs per tile; the PE contracts the remaining (s, slab)
     axes with fp16xfp16 products accumulated exactly in fp32 PSUM (M
     hi/lo stationary). Verified argmax-exact on the reference inputs with
     a 2.2e-3 worst-row margin (~100x the device-vs-host numeric noise).
  3. sim lands PSUM-transposed as two [c, (slab v)] banks; slab-reduces +
     identity-matmul transposes (exact: multiplies by 1.0/0.0) accumulate
     [v, c] into one PSUM tile for the row-max + is_equal one-hot.

Sharding: V is split across the 8 cores; every stage after the split is
core-local (no collectives). Per-core time is DMA-bound: ~8.9 MB/core
(x 8.4 MB fp16 + M 0.26 MB) at the ~358 GB/s HBM roofline, with the DVE
trees (~17us) and all PE work hidden under the stream.
"""

import sys

for _p in ("/opt/trn_rl_repo",):
    if _p not in sys.path:
        sys.path.append(_p)

from contextlib import ExitStack

import numpy as np

import concourse.bacc as bacc
import concourse.bass as bass
import concourse.mybir as mybir
from concourse import tile
from concourse.bass_utils import run_bass_kernel_spmd

B, S, V, H, C = 64, 1024, 512, 512, 64
NCORES = 8
VL = V // NCORES  # 64 V-columns per core
P = 128
ST = S // P  # 8 s-chunks
FH = (B // 2) * VL  # 2048 free elems per half-chunk tile
JH = 8  # slabs per half-tile left for the PE after the 2-level DVE tree
F16 = mybir.dt.float16
F32 = mybir.dt.float32

_NC_CACHE = None


def build_bass() -> bass.Bass:
    nc = bacc.Bacc("TRN2", target_bir_lowering=False)

    xs = nc.declare_dram_parameter("xs", [S, 2 * FH], F16, isOutput=False)
    mm = nc.declare_dram_parameter("mm", [P, 2 * ST * C], F16, isOutput=False)
    c16 = nc.declare_dram_parameter("c16", [1, VL + 2 * C], F16, isOutput=False)
    c32 = nc.declare_dram_parameter("c32", [C, C], F32, isOutput=False)
    out = nc.declare_dram_parameter("out", [VL, C], F32, isOutput=True)

    with tile.TileContext(nc) as tc, ExitStack() as ctx:
        consts = ctx.enter_context(tc.tile_pool(name="consts", bufs=1))
        xpool = ctx.enter_context(tc.tile_pool(name="x", bufs=16))
        spool = ctx.enter_context(tc.tile_pool(name="small", bufs=1))
        pst = ctx.enter_context(tc.tile_pool(name="pst", bufs=1, space="PSUM"))
        psca = ctx.enter_context(tc.tile_pool(name="psca", bufs=1, space="PSUM"))

        # const tiles; their DMAs ride the SP ring just behind the first
        # chunk (a second ring steals SDMA packet slots and slows the
        # stream). Flat 2D transfers only.
        msb = consts.tile([P, 2 * ST * C], F16)
        c16t = consts.tile([1, VL + 2 * C], F16)
        idt = consts.tile([C, C], F32)

        # final sim [v, c]; single PE-side accumulator [c, (slab v)] — both
        # b-halves overlay the same slab columns (their sums just add)
        psT = pst.tile([VL, C], F32, tag="psT")
        psC = psca.tile([C, JH * VL], F32, tag="psC")

        xs_r = xs.rearrange("(t p) f -> t p f", p=P)
        for t in range(ST):
            xhs = []
            for h in range(2):
                xh = xpool.tile([P, FH], F16, tag="xh")
                nc.sync.dma_start(
                    out=xh[:], in_=xs_r[t][:, h * FH : (h + 1) * FH]
                )
                # 2-level halving tree over contiguous column blocks
                # (fp16 2x mode); cols = b_local*VL + v
                nb = FH
                while nb > JH * VL:
                    hb = nb // 2
                    nc.vector.tensor_tensor(
                        xh[:, 0:hb], xh[:, 0:hb], xh[:, hb:nb],
                        op=mybir.AluOpType.add,
                    )
                    nb = hb
                xhs.append(xh)
            if t == 0:
                # consts land behind chunk 0 on the SP ring (a second ring
                # steals SDMA packet slots); the PE has slack to wait
                nc.sync.dma_start(out=msb[:], in_=mm[:])
                nc.sync.dma_start(out=c16t[:], in_=c16[:])
                nc.sync.dma_start(out=idt[:], in_=c32[:])
            # slab contraction, M_t hi/lo stationary (flat col slices)
            for h in range(2):
                for li in range(2):
                    nc.tensor.matmul(
                        psC[:],
                        msb[:, (li * ST + t) * C : (li * ST + t + 1) * C],
                        xhs[h][:, 0 : JH * VL],
                        start=(t == 0 and h == 0 and li == 0),
                        stop=(t == ST - 1 and h == 1 and li == 1),
                    )
            if t == 0:
                # bias early, off the critical path (psT group start)
                ones16 = c16t[0:1, 0:VL]
                nc.tensor.matmul(
                    psT[:], ones16, c16t[0:1, VL : VL + C],
                    start=True, stop=False,
                )
                nc.tensor.matmul(
                    psT[:], ones16, c16t[0:1, VL + C : VL + 2 * C],
                    start=False, stop=False,
                )

        # --- tail: slab-reduce, transpose into [v, c], argmax --------------
        sC = spool.tile([C, VL], F32, tag="sC")
        nc.vector.tensor_reduce(
            sC[:],
            psC[:].rearrange("c (s v) -> c v s", s=JH),
            axis=mybir.AxisListType.X,
            op=mybir.AluOpType.add,
        )
        nc.tensor.matmul(psT[:], sC[:], idt[:], start=False, stop=True)

        mx = spool.tile([VL, 1], F32)
        nc.vector.tensor_reduce(
            mx[:], psT[:], axis=mybir.AxisListType.X, op=mybir.AluOpType.max
        )
        oh = spool.tile([VL, C], F32)
        nc.vector.tensor_scalar(
            oh[:], psT[:], mx[:], None, op0=mybir.AluOpType.is_equal
        )
        nc.sync.dma_start(out=out[:], in_=oh[:])

    nc.compile()
    return nc


def _get_nc() -> bass.Bass:
    global _NC_CACHE
    if _NC_CACHE is None:
        _NC_CACHE = build_bass()
    return _NC_CACHE


def make_in_maps(x, W, b, centroids):
    x = np.asarray(x, dtype=np.float32)
    W = np.asarray(W, dtype=np.float32)
    b = np.asarray(b, dtype=np.float32)
    centroids = np.asarray(centroids, dtype=np.float32)

    # x-independent folds, in float64, shipped as exact fp16 hi+lo pairs
    cn = centroids.astype(np.float64)
    cn /= np.linalg.norm(cn, axis=1, keepdims=True)
    M = W.astype(np.float64).T @ cn.T  # [S, C]
    bn = np.float64(B) * (b.astype(np.float64) @ cn.T)  # [C]

    Mhi = M.astype(np.float16)
    Mlo = (M - Mhi.astype(np.float64)).astype(np.float16)
    mhost = np.empty((P, 2, ST, C), np.float16)
    mhost[:, 0] = Mhi.reshape(ST, P, C).transpose(1, 0, 2)
    mhost[:, 1] = Mlo.reshape(ST, P, C).transpose(1, 0, 2)
    mhost = np.ascontiguousarray(mhost).reshape(P, 2 * ST * C)

    bnhi = bn.astype(np.float16)
    bnlo = (bn - bnhi.astype(np.float64)).astype(np.float16)
    c16host = np.concatenate(
        [np.ones(VL, np.float16), bnhi, bnlo]
    ).reshape(1, VL + 2 * C)
    c32host = np.eye(C, dtype=np.float32)

    # Host layout [B,S,V] -> [S, B, VL] per core, in fp16 (cast first so the
    # transpose moves half the bytes). One pass to [S, B, V] (contiguous 1KB
    # runs), then a contiguous per-core V-slice.
    x16 = x.astype(np.float16)
    xsb = np.ascontiguousarray(x16.transpose(1, 0, 2))  # [S, B, V]
    in_maps = []
    for i in range(NCORES):
        xs_i = np.ascontiguousarray(
            xsb[:, :, i * VL : (i + 1) * VL]
        ).reshape(S, 2 * FH)
        in_maps.append(
            {"xs": xs_i, "mm": mhost, "c16": c16host, "c32": c32host}
        )
    return in_maps


def run(inputs: dict, trace: bool = False):
    """Run on the 8 NeuronCores; returns (full_output, BassKernelResults)."""
    nc = _get_nc()
    in_maps = make_in_maps(**inputs)
    res = run_bass_kernel_spmd(nc, in_maps, list(range(NCORES)), trace=trace)
    full = np.concatenate([r["out"] for r in res.results], axis=0)
    return full, res


def kernel(x, W, b, centroids) -> np.ndarray:
    full, _ = run({"x": x, "W": W, "b": b, "centroids": centroids})
    return full


# revision 26
# speedup vs baseline: 1.1851x; 1.0244x over previous
"""HardClusterAssigner Trainium2 kernel.

Reference computation:
    x_emb = mean_b(einsum('bsv,hs->bvh', x, W) + b)   # [V, H]
    assignments = one_hot(argmin(-l2norm(x_emb) @ l2norm(centroids).T))

Key transformations:
  1. mean over B commutes with the linear contraction over S, the l2norm of
     the embedding is a positive per-row scale (argmin-invariant), and the
     1/B + bias fold in exactly:
         sim[v,c] = (sum_b x)[s,v] @ M[s,c] + bn[c],
         M = W.T @ cn.T,  bn = B * (b @ cn.T),  cn = l2norm(centroids)
     M/bn are x-independent and folded on the host (fp64), shipped as exact
     fp16 hi+lo pairs (pair error ~1e-7 relative).
  2. x streams as fp16 (halves the dominant HBM traffic) in [s, b, v]
     layout as 16 half-chunk tiles on the SP HWDGE ring (consts ride the
     ACT ring in parallel). Each tile's b-reduction runs as a 2-level add tree
     on the DVE over fully contiguous 2D slices (fp16 2x packed mode),
     leaving 8 slabs per tile; the PE contracts the remaining (s, slab)
     axes with fp16xfp16 products accumulated exactly in fp32 PSUM (M
     hi/lo stationary). Verified argmax-exact on the reference inputs with
     a 2.2e-3 worst-row margin (~100x the device-vs-host numeric noise).
  3. sim lands PSUM-transposed as two [c, (slab v)] banks; slab-reduces +
     identity-matmul transposes (exact: multiplies by 1.0/0.0) accumulate
     [v, c] into one PSUM tile for the row-max + is_equal one-hot.

Sharding: V is split across the 8 cores; every stage after the split is
core-local (no collectives). Per-core time is DMA-bound: ~8.9 MB/core
(x 8.4 MB fp16 + M 0.26 MB) at the ~358 GB/s HBM roofline, with the DVE
trees (~17us) and all PE work hidden under the stream.
"""

import sys

for _p in ("/opt/trn_rl_repo",):
    if _p not in sys.path:
        sys.path.append(_p)

from contextlib import ExitStack

import numpy as np

import concourse.bacc as bacc
import concourse.bass as bass
import concourse.mybir as mybir
from concourse import tile
from concourse.bass_utils import run_bass_kernel_spmd

B, S, V, H, C = 64, 1024, 512, 512, 64
NCORES = 8
VL = V // NCORES  # 64 V-columns per core
P = 128
ST = S // P  # 8 s-chunks
FH = (B // 2) * VL  # 2048 free elems per half-chunk tile
JH = 8  # slabs per half-tile left for the PE after the 2-level DVE tree
F16 = mybir.dt.float16
F32 = mybir.dt.float32

_NC_CACHE = None


def build_bass() -> bass.Bass:
    nc = bacc.Bacc("TRN2", target_bir_lowering=False)

    xs = nc.declare_dram_parameter("xs", [S, 2 * FH], F16, isOutput=False)
    mm = nc.declare_dram_parameter("mm", [P, 2 * ST * C], F16, isOutput=False)
    c16 = nc.declare_dram_parameter("c16", [1, VL + 2 * C], F16, isOutput=False)
    c32 = nc.declare_dram_parameter("c32", [C, C], F32, isOutput=False)
    out = nc.declare_dram_parameter("out", [VL, C], F32, isOutput=True)

    with tile.TileContext(nc) as tc, ExitStack() as ctx:
        consts = ctx.enter_context(tc.tile_pool(name="consts", bufs=1))
        xpool = ctx.enter_context(tc.tile_pool(name="x", bufs=16))
        spool = ctx.enter_context(tc.tile_pool(name="small", bufs=1))
        pst = ctx.enter_context(tc.tile_pool(name="pst", bufs=1, space="PSUM"))
        psca = ctx.enter_context(tc.tile_pool(name="psca", bufs=1, space="PSUM"))

        # const tiles; their DMAs ride the SP ring just behind the first
        # chunk (a second ring steals SDMA packet slots and slows the
        # stream). Flat 2D transfers only.
        msb = consts.tile([P, 2 * ST * C], F16)
        c16t = consts.tile([1, VL + 2 * C], F16)
        idt = consts.tile([C, C], F32)

        # final sim [v, c]; single PE-side accumulator [c, (slab v)] — both
        # b-halves overlay the same slab columns (their sums just add)
        psT = pst.tile([VL, C], F32, tag="psT")
        psC = psca.tile([C, JH * VL], F32, tag="psC")

        xs_r = xs.rearrange("(t p) f -> t p f", p=P)
        for t in range(ST):
            # middle chunks run 1-level trees (16 slabs, extra PE segment)
            # to keep the DVE well under the stream rate; edge chunks run
            # 2-level (light PE while cold at the head, short tail chain)
            nlvl = 1 if 2 <= t <= 5 else 2
            xhs = []
            for h in range(2):
                xh = xpool.tile([P, FH], F16, tag="xh")
                nc.sync.dma_start(
                    out=xh[:], in_=xs_r[t][:, h * FH : (h + 1) * FH]
                )
                # halving add tree over contiguous column blocks (fp16 2x
                # mode); cols = b_local*VL + v
                nb = FH
                for _ in range(nlvl):
                    hb = nb // 2
                    nc.vector.tensor_tensor(
                        xh[:, 0:hb], xh[:, 0:hb], xh[:, hb:nb],
                        op=mybir.AluOpType.add,
                    )
                    nb = hb
                xhs.append((xh, nb))
            if t == 0:
                # consts land behind chunk 0 on the SP ring (a second ring
                # steals SDMA packet slots); the PE has slack to wait
                nc.sync.dma_start(out=msb[:], in_=mm[:])
                nc.sync.dma_start(out=c16t[:], in_=c16[:])
                nc.sync.dma_start(out=idt[:], in_=c32[:])
            # slab contraction, M_t hi/lo stationary; slab segments beyond
            # the first overlay the same psC columns (sums just accumulate)
            for h in range(2):
                xh, nb = xhs[h]
                for li in range(2):
                    nseg = nb // (JH * VL)
                    for g in range(nseg):
                        nc.tensor.matmul(
                            psC[:],
                            msb[:, (li * ST + t) * C : (li * ST + t + 1) * C],
                            xh[:, g * JH * VL : (g + 1) * JH * VL],
                            start=(t == 0 and h == 0 and li == 0 and g == 0),
                            stop=(
                                t == ST - 1 and h == 1 and li == 1
                                and g == nseg - 1
                            ),
                        )
            if t == 0:
                # bias forced early, off the tail critical path
                ones16 = c16t[0:1, 0:VL]
                with tc.high_priority():
                    nc.tensor.matmul(
                        psT[:], ones16, c16t[0:1, VL : VL + C],
                        start=True, stop=False,
                    )
                    nc.tensor.matmul(
                        psT[:], ones16, c16t[0:1, VL + C : VL + 2 * C],
                        start=False, stop=False,
                    )

        # --- tail: slab-reduce, transpose into [v, c], argmax --------------
        sC = spool.tile([C, VL], F32, tag="sC")
        nc.vector.tensor_reduce(
            sC[:],
            psC[:].rearrange("c (s v) -> c v s", s=JH),
            axis=mybir.AxisListType.X,
            op=mybir.AluOpType.add,
        )
        nc.tensor.matmul(psT[:], sC[:], idt[:], start=False, stop=True)

        mx = spool.tile([VL, 1], F32)
        nc.vector.tensor_reduce(
            mx[:], psT[:], axis=mybir.AxisListType.X, op=mybir.AluOpType.max
        )
        oh = spool.tile([VL, C], F32)
        nc.vector.tensor_scalar(
            oh[:], psT[:], mx[:], None, op0=mybir.AluOpType.is_equal
        )
        nc.sync.dma_start(out=out[:], in_=oh[:])

    nc.compile()
    return nc


def _get_nc() -> bass.Bass:
    global _NC_CACHE
    if _NC_CACHE is None:
        _NC_CACHE = build_bass()
    return _NC_CACHE


def make_in_maps(x, W, b, centroids):
    x = np.asarray(x, dtype=np.float32)
    W = np.asarray(W, dtype=np.float32)
    b = np.asarray(b, dtype=np.float32)
    centroids = np.asarray(centroids, dtype=np.float32)

    # x-independent folds, in float64, shipped as exact fp16 hi+lo pairs
    cn = centroids.astype(np.float64)
    cn /= np.linalg.norm(cn, axis=1, keepdims=True)
    M = W.astype(np.float64).T @ cn.T  # [S, C]
    bn = np.float64(B) * (b.astype(np.float64) @ cn.T)  # [C]

    Mhi = M.astype(np.float16)
    Mlo = (M - Mhi.astype(np.float64)).astype(np.float16)
    mhost = np.empty((P, 2, ST, C), np.float16)
    mhost[:, 0] = Mhi.reshape(ST, P, C).transpose(1, 0, 2)
    mhost[:, 1] = Mlo.reshape(ST, P, C).transpose(1, 0, 2)
    mhost = np.ascontiguousarray(mhost).reshape(P, 2 * ST * C)

    bnhi = bn.astype(np.float16)
    bnlo = (bn - bnhi.astype(np.float64)).astype(np.float16)
    c16host = np.concatenate(
        [np.ones(VL, np.float16), bnhi, bnlo]
    ).reshape(1, VL + 2 * C)
    c32host = np.eye(C, dtype=np.float32)

    # Host layout [B,S,V] -> [S, B, VL] per core, in fp16 (cast first so the
    # transpose moves half the bytes). One pass to [S, B, V] (contiguous 1KB
    # runs), then a contiguous per-core V-slice.
    x16 = x.astype(np.float16)
    xsb = np.ascontiguousarray(x16.transpose(1, 0, 2))  # [S, B, V]
    in_maps = []
    for i in range(NCORES):
        xs_i = np.ascontiguousarray(
            xsb[:, :, i * VL : (i + 1) * VL]
        ).reshape(S, 2 * FH)
        in_maps.append(
            {"xs": xs_i, "mm": mhost, "c16": c16host, "c32": c32host}
        )
    return in_maps


def run(inputs: dict, trace: bool = False):
    """Run on the 8 NeuronCores; returns (full_output, BassKernelResults)."""
    nc = _get_nc()
    in_maps = make_in_maps(**inputs)
    res = run_bass_kernel_spmd(nc, in_maps, list(range(NCORES)), trace=trace)
    full = np.concatenate([r["out"] for r in res.results], axis=0)
    return full, res


def kernel(x, W, b, centroids) -> np.ndarray:
    full, _ = run({"x": x, "W": W, "b": b, "centroids": centroids})
    return full
